# revision 14
# baseline (speedup 1.0000x reference)
"""Window-routed sparse attention on 8 TRN2 NeuronCores.

Sharding: 64 windows x 8 cores = 8 windows/core (embarrassingly parallel).
Host precomputes the tiny routing path (region means, a_r [64,64]) and the
window-mixed q_m/k_m in fp32; each core runs the heavy windowed attention
relu(q_m k_m^T) v for its 8 windows on the Tensor engine in bf16 (f32 PSUM
accumulation). Relu alternates between the Scalar and Vector engines so it
hides under the matmuls; PSUM->SBUF output copies run on GpSimd.
"""

import sys

sys.path.insert(0, "/opt/trn_rl_repo")

import numpy as np
import ml_dtypes

C = 64          # channels
NW = 64         # windows (8x8 grid of 32x32 patches on 256x256)
T = 1024        # tokens per window (32*32)
NCORES = 8
WPC = NW // NCORES  # windows per core
BF16 = ml_dtypes.bfloat16

_CACHE = {}


def _build_program():
    import concourse.mybir as mybir
    from concourse import bacc
    from concourse.tile import TileContext

    bf16 = mybir.dt.bfloat16
    f32 = mybir.dt.float32

    nc = bacc.Bacc(None, target_bir_lowering=False)
    # c-major [c, i, t] for q_m/k_m; [s, i, k, c] for v (s = token % 128,
    # k = token // 128 within the window)
    qm_d = nc.declare_dram_parameter("qm", [C, WPC, T], bf16, isOutput=False)
    km_d = nc.declare_dram_parameter("km", [C, WPC, T], bf16, isOutput=False)
    v_d = nc.declare_dram_parameter("v", [128, WPC, 8, C], bf16, isOutput=False)
    o_d = nc.declare_dram_parameter("o", [C, WPC, T], bf16, isOutput=True)

    with TileContext(nc) as tc:
        with (
            tc.tile_pool(name="in", bufs=1) as in_pool,
            tc.tile_pool(name="at", bufs=3) as a_pool,
            tc.tile_pool(name="ob", bufs=2) as o_pool,
            tc.tile_pool(name="pa", bufs=3, space="PSUM") as pa_pool,
            tc.tile_pool(name="po", bufs=1, space="PSUM") as po_pool,
        ):
            # persistent SBUF tiles, one set per window so each window's
            # compute waits only on its own three DMAs
            qm_w, km_w, v_w, o_w = [], [], [], []
            for i in range(WPC):
                qm_i = in_pool.tile([C, T], bf16, tag=f"qm{i}", name=f"qm{i}")
                km_i = in_pool.tile([C, T], bf16, tag=f"km{i}", name=f"km{i}")
                v_i = in_pool.tile([128, 8, C], bf16, tag=f"v{i}", name=f"v{i}")
                o_i = in_pool.tile([C, T], bf16, tag=f"o{i}", name=f"o{i}")
                nc.sync.dma_start(out=qm_i, in_=qm_d[:, i])
                nc.sync.dma_start(out=km_i, in_=km_d[:, i])
                nc.sync.dma_start(out=v_i, in_=v_d[:, i])
                qm_w.append(qm_i); km_w.append(km_i)
                v_w.append(v_i); o_w.append(o_i)

            # PE warm-up: dummy matmuls on scratch SBUF while the input DMAs
            # land. Keeps the tensor engine busy from t=0 so its clock is at
            # the top p-state when real data arrives.
            warm_s = in_pool.tile([C, 512], bf16, tag="warm", name="warm_s")
            nc.gpsimd.memset(warm_s, 0)
            for w in range(16):
                warm_p = pa_pool.tile([128, 512], f32, tag="psa", name="warm_p")
                nc.tensor.matmul(
                    out=warm_p,
                    lhsT=warm_s[:, 0:128],
                    rhs=warm_s,
                    start=True,
                    stop=True,
                )

            for i in range(WPC):
                # Software-pipelined: QK for s-chunk k+2 issues before AV for
                # chunk k, so the PE never waits on a relu. Relu halves run
                # concurrently on Scalar (t 0:512) and Vector (t 512:1024).
                ps_o = po_pool.tile([C, T], f32, tag="pso")
                attn = {}

                def emit_qk(k):
                    ps_a = pa_pool.tile([128, T], f32, tag="psa", name="psa")
                    at = a_pool.tile([128, T], bf16, tag="attn", name="attn")
                    for h in range(2):
                        nc.tensor.matmul(
                            out=ps_a[:, h * 512:(h + 1) * 512],
                            lhsT=km_w[i][:, k * 128:(k + 1) * 128],
                            rhs=qm_w[i][:, h * 512:(h + 1) * 512],
                            start=True,
                            stop=True,
                        )
                    nc.scalar.activation(
                        out=at[:, 0:512],
                        in_=ps_a[:, 0:512],
                        func=mybir.ActivationFunctionType.Relu,
                        scale=1.0,
                    )
                    nc.vector.tensor_scalar_max(at[:, 512:1024], ps_a[:, 512:1024], 0.0)
                    attn[k] = at

                emit_qk(0)
                emit_qk(1)
                for k in range(8):
                    if k + 2 < 8:
                        emit_qk(k + 2)
                    at = attn.pop(k)
                    for h in range(2):
                        nc.tensor.matmul(
                            out=ps_o[:, h * 512:(h + 1) * 512],
                            lhsT=v_w[i][:, k, :],
                            rhs=at[:, h * 512:(h + 1) * 512],
                            start=(k == 0),
                            stop=(k == 7),
                        )
                # split PSUM->SBUF output copy across both free engines
                nc.scalar.activation(
                    out=o_w[i][:, 0:512],
                    in_=ps_o[:, 0:512],
                    func=mybir.ActivationFunctionType.Copy,
                    scale=1.0,
                )
                nc.vector.tensor_copy(out=o_w[i][:, 512:1024], in_=ps_o[:, 512:1024])
                nc.sync.dma_start(out=o_d[:, i], in_=o_w[i])

    nc.finalize()
    return nc


def kernel(x, W, bias, _trace=False):
    global LAST_RESULT
    from concourse.bass_utils import run_bass_kernel_spmd

    x = np.asarray(x, dtype=np.float32)
    W = np.asarray(W, dtype=np.float32)
    bias = np.asarray(bias, dtype=np.float32)

    # ---- host prep: windows, qkv, routing, mixing (tiny vs attention) ----
    # xw: [nw, T, c]
    xw = (
        x.reshape(C, 8, 32, 8, 32)
        .transpose(1, 3, 2, 4, 0)
        .reshape(NW, T, C)
    )
    qkv = xw @ W.T + bias  # [nw, T, 3c]
    q, k, v = qkv[..., :C], qkv[..., C:2 * C], qkv[..., 2 * C:]
    q_r = q.mean(axis=1)  # [nw, c]
    k_r = k.mean(axis=1)
    a_r = np.maximum(q_r @ k_r.T, 0.0)  # [nw, nw]
    k_m = np.tensordot(a_r, k, axes=(1, 0))  # [nw, T, c]
    q_m = np.tensordot(a_r, q, axes=(1, 0))

    if "nc" not in _CACHE:
        _CACHE["nc"] = _build_program()
    nc = _CACHE["nc"]

    in_maps = []
    for m in range(NCORES):
        s = slice(m * WPC, (m + 1) * WPC)
        # v: [wpc, T, c] -> [wpc, k, s(128), c] -> [s, wpc, k, c]
        v_s = v[s].reshape(WPC, 8, 128, C).transpose(2, 0, 1, 3)
        in_maps.append({
            "qm": np.ascontiguousarray(q_m[s].transpose(2, 0, 1)).astype(BF16),
            "km": np.ascontiguousarray(k_m[s].transpose(2, 0, 1)).astype(BF16),
            "v": np.ascontiguousarray(v_s).astype(BF16),
        })

    res = run_bass_kernel_spmd(nc, in_maps, list(range(NCORES)), trace=_trace)
    LAST_RESULT = res
    outs = [
        res.results[m]["o"].astype(np.float32).reshape(C, WPC, T)
        for m in range(NCORES)
    ]
    o_cm = np.concatenate(outs, axis=1)  # [c, nw, T]

    # fold back: [c, jh, jw, th, tw] -> [1, c, 256, 256]
    o_img = (
        o_cm.reshape(C, 8, 8, 32, 32)
        .transpose(0, 1, 3, 2, 4)
        .reshape(1, C, 256, 256)
    )
    return o_img.astype(np.float32)


LAST_RESULT = None  # BassKernelResults from the most recent run (for test.py)


# revision 15
# speedup vs baseline: 1.3668x; 1.3668x over previous
"""Window-routed sparse attention on 8 TRN2 NeuronCores.

Sharding: 64 windows x 8 cores = 8 windows/core (embarrassingly parallel).
Host precomputes the tiny routing path (region means, a_r [64,64]) and the
window-mixed q_m/k_m in fp32; each core runs the heavy windowed attention
relu(q_m k_m^T) v for its 8 windows:

  - QK^T in bf16 (f32 PSUM), software-pipelined so the PE never waits
  - relu runs on Scalar (even chunks) and Vector (odd chunks) engines,
    emitting scaled fp8e4 attention weights
  - attn @ v in fp8e4 with DoubleRow perf mode (2x matmul throughput),
    accumulating 256 contraction rows per step in f32 PSUM
"""

import sys

sys.path.insert(0, "/opt/trn_rl_repo")

import numpy as np
import ml_dtypes

C = 64          # channels
NW = 64         # windows (8x8 grid of 32x32 patches on 256x256)
T = 1024        # tokens per window (32*32)
NCORES = 8
WPC = NW // NCORES  # windows per core
BF16 = ml_dtypes.bfloat16

_CACHE = {}


def _build_program(s_a):
    import concourse.mybir as mybir
    from concourse import bacc
    from concourse.tile import TileContext

    bf16 = mybir.dt.bfloat16
    f32 = mybir.dt.float32
    f8 = mybir.dt.float8e4

    nc = bacc.Bacc(None, target_bir_lowering=False)
    # c-major [c, i, t] for q_m/k_m; v pre-packed for DoubleRow:
    # v8[p, i, kk, j, c] = v[i, 256*kk + 128*j + p, c] (scaled to fp8)
    qm_d = nc.declare_dram_parameter("qm", [C, WPC, T], bf16, isOutput=False)
    km_d = nc.declare_dram_parameter("km", [C, WPC, T], bf16, isOutput=False)
    v_d = nc.declare_dram_parameter("v", [128, WPC, 4, 2, C], f8, isOutput=False)
    o_d = nc.declare_dram_parameter("o", [C, WPC, T], bf16, isOutput=True)

    with TileContext(nc) as tc:
        with (
            tc.tile_pool(name="in", bufs=1) as in_pool,
            tc.tile_pool(name="at", bufs=3) as a_pool,
            tc.tile_pool(name="pa", bufs=3, space="PSUM") as pa_pool,
            tc.tile_pool(name="po", bufs=1, space="PSUM") as po_pool,
        ):
            # persistent SBUF tiles, one set per window so each window's
            # compute waits only on its own three DMAs
            qm_w, km_w, v_w, o_w = [], [], [], []
            for i in range(WPC):
                qm_i = in_pool.tile([C, T], bf16, tag=f"qm{i}", name=f"qm{i}")
                km_i = in_pool.tile([C, T], bf16, tag=f"km{i}", name=f"km{i}")
                v_i = in_pool.tile([128, 4, 2, C], f8, tag=f"v{i}", name=f"v{i}")
                o_i = in_pool.tile([C, T], bf16, tag=f"o{i}", name=f"o{i}")
                nc.sync.dma_start(out=qm_i, in_=qm_d[:, i])
                nc.sync.dma_start(out=km_i, in_=km_d[:, i])
                nc.sync.dma_start(out=v_i, in_=v_d[:, i])
                qm_w.append(qm_i); km_w.append(km_i)
                v_w.append(v_i); o_w.append(o_i)

            for i in range(WPC):
                # pipelined per window: QK/relu for chunk pair kk+1 issue
                # before the fp8 DoubleRow AV for pair kk
                ps_o = po_pool.tile([C, T], f32, tag="pso")
                at_live = {}

                def emit_pair(kk):
                    at8 = a_pool.tile([128, 2, T], f8, tag="attn", name="attn")
                    for j in range(2):
                        k = 2 * kk + j
                        ps_a = pa_pool.tile([128, T], f32, tag="psa", name="psa")
                        for h in range(2):
                            nc.tensor.matmul(
                                out=ps_a[:, h * 512:(h + 1) * 512],
                                lhsT=km_w[i][:, k * 128:(k + 1) * 128],
                                rhs=qm_w[i][:, h * 512:(h + 1) * 512],
                                start=True,
                                stop=True,
                            )
                        if j == 0:
                            nc.scalar.activation(
                                out=at8[:, 0, :],
                                in_=ps_a,
                                func=mybir.ActivationFunctionType.Relu,
                                scale=float(s_a),
                            )
                        else:
                            nc.vector.tensor_scalar(
                                out=at8[:, 1, :],
                                in0=ps_a,
                                scalar1=float(s_a),
                                scalar2=0.0,
                                op0=mybir.AluOpType.mult,
                                op1=mybir.AluOpType.max,
                            )
                    at_live[kk] = at8

                emit_pair(0)
                for kk in range(4):
                    if kk + 1 < 4:
                        emit_pair(kk + 1)
                    at8 = at_live.pop(kk)
                    for h in range(2):
                        nc.tensor.matmul(
                            out=ps_o[:, h * 512:(h + 1) * 512],
                            lhsT=v_w[i][:, kk],
                            rhs=at8[:, :, h * 512:(h + 1) * 512],
                            perf_mode=mybir.MatmulPerfMode.DoubleRow,
                            start=(kk == 0),
                            stop=(kk == 3),
                        )
                # split PSUM->SBUF output copy across both free engines
                nc.scalar.activation(
                    out=o_w[i][:, 0:512],
                    in_=ps_o[:, 0:512],
                    func=mybir.ActivationFunctionType.Copy,
                    scale=1.0,
                )
                nc.vector.tensor_copy(out=o_w[i][:, 512:1024], in_=ps_o[:, 512:1024])
                nc.sync.dma_start(out=o_d[:, i], in_=o_w[i])

    nc.finalize()
    return nc


def kernel(x, W, bias, _trace=False):
    global LAST_RESULT
    from concourse.bass_utils import run_bass_kernel_spmd
    import concourse.mybir as mybir

    E4 = mybir.dt.np(mybir.dt.float8e4)

    x = np.asarray(x, dtype=np.float32)
    W = np.asarray(W, dtype=np.float32)
    bias = np.asarray(bias, dtype=np.float32)

    # ---- host prep: windows, qkv, routing, mixing (tiny vs attention) ----
    xw = (
        x.reshape(C, 8, 32, 8, 32)
        .transpose(1, 3, 2, 4, 0)
        .reshape(NW, T, C)
    )
    qkv = xw @ W.T + bias  # [nw, T, 3c]
    q, k, v = qkv[..., :C], qkv[..., C:2 * C], qkv[..., 2 * C:]
    q_r = q.mean(axis=1)  # [nw, c]
    k_r = k.mean(axis=1)
    a_r = np.maximum(q_r @ k_r.T, 0.0)  # [nw, nw]
    k_m = np.tensordot(a_r, k, axes=(1, 0))  # [nw, T, c]
    q_m = np.tensordot(a_r, q, axes=(1, 0))

    # fp8 scales: bound attn logits via Cauchy-Schwarz, v by its max
    bound = max(
        np.linalg.norm(q_m[i], axis=-1).max() * np.linalg.norm(k_m[i], axis=-1).max()
        for i in range(NW)
    )
    s_a = 240.0 / float(bound)
    s_v = 240.0 / float(np.abs(v).max())

    key = ("nc", round(float(s_a), 6))
    if key not in _CACHE:
        _CACHE.clear()
        _CACHE[key] = _build_program(s_a)
    nc = _CACHE[key]

    in_maps = []
    for m in range(NCORES):
        s = slice(m * WPC, (m + 1) * WPC)
        # v8[p, i, kk, j, c] = v[i, 256*kk + 128*j + p, c] * s_v
        v8 = (v[s].reshape(WPC, 4, 2, 128, C) * s_v).astype(E4).transpose(3, 0, 1, 2, 4)
        in_maps.append({
            "qm": np.ascontiguousarray(q_m[s].transpose(2, 0, 1)).astype(BF16),
            "km": np.ascontiguousarray(k_m[s].transpose(2, 0, 1)).astype(BF16),
            "v": np.ascontiguousarray(v8),
        })

    res = run_bass_kernel_spmd(nc, in_maps, list(range(NCORES)), trace=_trace)
    LAST_RESULT = res
    inv = 1.0 / (s_a * s_v)
    outs = [
        res.results[m]["o"].astype(np.float32).reshape(C, WPC, T) * inv
        for m in range(NCORES)
    ]
    o_cm = np.concatenate(outs, axis=1)  # [c, nw, T]

    # fold back: [c, jh, jw, th, tw] -> [1, c, 256, 256]
    o_img = (
        o_cm.reshape(C, 8, 8, 32, 32)
        .transpose(0, 1, 3, 2, 4)
        .reshape(1, C, 256, 256)
    )
    return o_img.astype(np.float32)


LAST_RESULT = None  # BassKernelResults from the most recent run (for test.py)
